# revision 5
# baseline (speedup 1.0000x reference)
"""Adaptive per-pixel LoG 9x9 convolution on 8 TRN2 NeuronCores.

Math: out[b,c,y,x] = sum_{dy,dx in [-4,4]} xpad[b,c,y+dy,x+dx] * K(dx^2+dy^2; p)
The kernel depends on the offset only through r2 = dx^2+dy^2, which takes 15
distinct values over the 9x9 window -> exact rank-15 decomposition:
    out[c,p] = sum_v w_v[p] * S_v[c,p]
where S_v = fixed "ring sum" convolutions (34 shifted vector adds via a
column-class/row-class sharing scheme instead of 81 MACs) and
    w_v[p] = (base[p] - r2_v * B2[p]) * exp(-r2_v * inv2s2[p])
with base = -dist*sqrt(sigma)/(pi*sigma^4), inv2s2 = 1/(2 sigma^2),
B2 = base*inv2s2 (tiny per-pixel scalar fields, prepared on host).

Sharding: 8 cores = 4 batches x 2 row-halves (128 output rows each). Within a
core, SBUF partition p = (row-strip si, col-block bi): a 16x16 output tile plus
4-pixel halo (24x24 input window, all 3 channels). All shifts become free-dim
AP offsets; host bakes the replicated window layout so every DMA is contiguous.

Perf: ring sums / products run in bf16 (DVE 2x_1p mode). A second 1-col-shifted
input copy (xp1) keeps every tap 4B-aligned so the 2x mode engages. exps on the
scalar engine overlap the vector work.  Products are tree-reduced to keep bf16
rounding depth shallow; final accumulate + output stay f32.
"""

import math

import numpy as np

B, C, H, W = 4, 3, 256, 256
PAD = 4
SIGMA_MIN, SIGMA_MAX = 0.5, 10.0
N_CORES = 8

# strip geometry: 8 row-strips x 16 col-blocks = 128 partitions per core
S_ROWS = 16
S_COLS = 16
N_STRIPS = 8
N_BLOCKS = 16
IN_R = S_ROWS + 2 * PAD  # 24
IN_C = S_COLS + 2 * PAD  # 24
IN_C1 = 22  # xp1 = cols +1..+22 of the window (only even-offset taps needed)

R2_VALUES = sorted({dx * dx + dy * dy for dx in range(-4, 5) for dy in range(-4, 5)})
assert len(R2_VALUES) == 15
NV = len(R2_VALUES)


def _ring_terms(v):
    """(d, dy) terms for ring v: U_d row-shifted by dy; d = |dx| column class."""
    terms = []
    for dy in range(-4, 5):
        a = v - dy * dy
        if a in (0, 1, 4, 9, 16):
            terms.append((int(math.isqrt(a)), dy))
    return terms


def _build_program(nc, bass, mybir):
    f32 = mybir.dt.float32
    bf16 = mybir.dt.bfloat16
    Alu = mybir.AluOpType
    Act = mybir.ActivationFunctionType

    xp_d = nc.declare_dram_parameter("xp", [128, C, IN_R, IN_C], bf16, isOutput=False)
    xp1_d = nc.declare_dram_parameter("xp1", [128, C, IN_R, IN_C1], bf16, isOutput=False)
    win_d = nc.declare_dram_parameter("win", [128, 3, S_ROWS, S_COLS], f32, isOutput=False)
    out_d = nc.declare_dram_parameter("out", [128, C, S_ROWS, S_COLS], f32, isOutput=True)

    with (
        nc.Block() as block,
        nc.semaphore("dma_sem") as dma_sem,
        nc.semaphore("win_sem") as win_sem,
        nc.semaphore("act_sem") as act_sem,
        nc.semaphore("dve_sem") as dve_sem,
        nc.sbuf_tensor("s_xp", [128, C, IN_R, IN_C], bf16) as xp,
        nc.sbuf_tensor("s_xp1", [128, C, IN_R, IN_C1], bf16) as xp1,
        nc.sbuf_tensor("s_win", [128, 3, S_ROWS, S_COLS], f32) as win,
        nc.sbuf_tensor("s_wb", [128, 2, S_ROWS, S_COLS], bf16) as wb,
        nc.sbuf_tensor("U", [128, 4, C, IN_R, S_COLS], bf16) as U,
        nc.sbuf_tensor("S", [128, NV, C, S_ROWS, S_COLS], bf16) as S,
        nc.sbuf_tensor("E", [128, 14, S_ROWS, S_COLS], bf16) as E,
        nc.sbuf_tensor("Wv", [128, 14, S_ROWS, S_COLS], bf16) as Wv,
        nc.sbuf_tensor("P", [128, NV, C, S_ROWS, S_COLS], bf16) as P,
        nc.sbuf_tensor("acc", [128, C, S_ROWS, S_COLS], f32) as acc,
    ):
        inv2s2 = win[:, 0]
        base_f = win[:, 1]
        b2_f = win[:, 2]
        base_b = wb[:, 0]
        b2_b = wb[:, 1]

        def u_view(d, dy):
            # U_d at output rows shifted by dy (strip rows 4+dy .. 19+dy)
            if d == 0:
                return xp[:, :, PAD + dy : PAD + dy + S_ROWS, PAD : PAD + S_COLS]
            return U[:, d - 1, :, PAD + dy : PAD + dy + S_ROWS, :]

        @block.sync
        def _(sync):
            sync.dma_start(out=xp[:], in_=xp_d[:]).then_inc(dma_sem, 16)
            sync.dma_start(out=xp1[:], in_=xp1_d[:]).then_inc(dma_sem, 16)
            sync.wait_ge(dve_sem, 1)
            sync.dma_start(out=out_d[:], in_=acc[:]).then_inc(dma_sem, 16)
            sync.wait_ge(dma_sem, 48)

        @block.scalar
        def _(scalar):
            scalar.dma_start(out=win[:], in_=win_d[:]).then_inc(win_sem, 16)
            scalar.wait_ge(win_sem, 16)
            # bf16 casts of base/B2 for the DVE's 2x-mode STT ops
            scalar.copy(base_b, base_f).then_inc(act_sem, 1)
            scalar.copy(b2_b, b2_f).then_inc(act_sem, 1)
            for i, v in enumerate(R2_VALUES[1:]):
                scalar.activation(
                    E[:, i], inv2s2, Act.Exp, bias=0.0, scale=float(-v)
                ).then_inc(act_sem, 1)

        @block.vector
        def _(vector):
            # w-gen part 1: Wv[i] = base - r2_v * B2  (bf16, 2x)
            vector.wait_ge(act_sem, 2)
            for i, v in enumerate(R2_VALUES[1:]):
                vector.scalar_tensor_tensor(
                    Wv[:, i], b2_b, float(-v), base_b, Alu.mult, Alu.add
                )

            # stage 1: column-class sums U_d over all strip rows
            # (xp1 supplies the odd shifts so every AP stays 4B-aligned)
            vector.wait_ge(dma_sem, 32)
            # U_1[x] = xp[x+3] + xp[x+5] = xp1[x+2] + xp1[x+4]
            vector.tensor_tensor(
                U[:, 0], xp1[:, :, :, 2 : 2 + S_COLS], xp1[:, :, :, 4 : 4 + S_COLS], Alu.add
            )
            # U_2[x] = xp[x+2] + xp[x+6]
            vector.tensor_tensor(
                U[:, 1], xp[:, :, :, 2 : 2 + S_COLS], xp[:, :, :, 6 : 6 + S_COLS], Alu.add
            )
            # U_3[x] = xp[x+1] + xp[x+7] = xp1[x] + xp1[x+6]
            vector.tensor_tensor(
                U[:, 2], xp1[:, :, :, 0:S_COLS], xp1[:, :, :, 6 : 6 + S_COLS], Alu.add
            )
            # U_4[x] = xp[x] + xp[x+8]
            vector.tensor_tensor(
                U[:, 3], xp[:, :, :, 0:S_COLS], xp[:, :, :, 8 : 8 + S_COLS], Alu.add
            )

            # stage 2: ring sums S_v
            s_views = {}
            for vi, v in enumerate(R2_VALUES):
                terms = _ring_terms(v)
                if len(terms) == 1:
                    d, dy = terms[0]
                    s_views[vi] = u_view(d, dy)
                    continue
                dst = S[:, vi]
                s_views[vi] = dst
                (d0, dy0), (d1, dy1) = terms[0], terms[1]
                vector.tensor_tensor(dst, u_view(d0, dy0), u_view(d1, dy1), Alu.add)
                for d, dy in terms[2:]:
                    vector.tensor_tensor(dst, dst, u_view(d, dy), Alu.add)

            # w-gen part 2: Wv[i] *= E[i]
            vector.wait_ge(act_sem, 16)
            for i in range(14):
                vector.tensor_tensor(Wv[:, i], Wv[:, i], E[:, i], Alu.mult)

            # products P_v = w_v * S_v (w broadcast across channels)
            def wv_bcast(ap2d):
                # [128, S_ROWS, S_COLS] -> [128, C(0-stride), S_ROWS*S_COLS]
                t = ap2d.tensor
                return bass.AP(t, ap2d.offset, [ap2d.ap[0], [0, C]] + list(ap2d.ap[1:]))

            vector.tensor_tensor(P[:, 0], s_views[0], wv_bcast(base_b), Alu.mult)
            for vi in range(1, NV):
                vector.tensor_tensor(P[:, vi], s_views[vi], wv_bcast(Wv[:, vi - 1]), Alu.mult)

            # tree-reduce the 15 products (shallow bf16 rounding depth)
            vector.tensor_tensor(P[:, 0:7], P[:, 0:7], P[:, 7:14], Alu.add)
            vector.tensor_tensor(P[:, 0:3], P[:, 0:3], P[:, 3:6], Alu.add)
            vector.tensor_tensor(P[:, 0], P[:, 0], P[:, 1], Alu.add)
            vector.tensor_tensor(P[:, 0], P[:, 0], P[:, 2], Alu.add)
            vector.tensor_tensor(P[:, 0], P[:, 0], P[:, 6], Alu.add)
            vector.tensor_tensor(P[:, 0], P[:, 0], P[:, 14], Alu.add)

            # upcast to f32 and ship out
            vector.tensor_copy(acc[:], P[:, 0]).then_inc(dve_sem, 1)

    return nc


_PROGRAM_CACHE = {}


def _get_program():
    if "nc" not in _PROGRAM_CACHE:
        import sys

        if "/opt/trn_rl_repo" not in sys.path:
            sys.path.insert(0, "/opt/trn_rl_repo")
        from concourse import bass, mybir

        nc = bass.Bass()
        _PROGRAM_CACHE["nc"] = _build_program(nc, bass, mybir)
    return _PROGRAM_CACHE["nc"]


def _host_prep(x, foa_xy):
    """Build per-core input maps (all layout gymnastics in numpy)."""
    import ml_dtypes

    bf = ml_dtypes.bfloat16
    xpad = np.pad(x, ((0, 0), (0, 0), (PAD, PAD), (PAD, PAD)), mode="reflect")
    xpad_bf = xpad.astype(bf)
    diag = math.sqrt(H * H + W * W)
    in_maps = []
    for core in range(N_CORES):
        b, half = divmod(core, 2)
        y0 = half * 128
        xph = xpad_bf[b, :, y0 : y0 + 136, :]  # [3, 136, 264]
        sw = np.lib.stride_tricks.sliding_window_view(xph, (C, IN_R, IN_C))
        XP = sw[0, ::S_ROWS, ::S_COLS]  # [8, 16, 3, 24, 24]
        XP = np.ascontiguousarray(XP.reshape(128, C, IN_R, IN_C))
        sw1 = np.lib.stride_tricks.sliding_window_view(xph, (C, IN_R, IN_C1))
        XP1 = sw1[0, ::S_ROWS, 1::S_COLS][:, :N_BLOCKS]  # windows at col bi*16+1
        XP1 = np.ascontiguousarray(XP1.reshape(128, C, IN_R, IN_C1))

        yy, xx = np.meshgrid(
            np.arange(y0, y0 + 128, dtype=np.float64),
            np.arange(W, dtype=np.float64),
            indexing="ij",
        )
        fx, fy = float(foa_xy[b, 0]), float(foa_xy[b, 1])
        dist = np.sqrt((xx - fx) ** 2 + (yy - fy) ** 2)
        dn = dist / diag
        sigma = (1.0 - dn) * SIGMA_MIN + dn * SIGMA_MAX
        inv2s2 = 1.0 / (2.0 * sigma * sigma)
        base = -dist * np.sqrt(sigma) / (math.pi * sigma**4)
        b2 = base * inv2s2
        WIN = np.stack([inv2s2, base, b2], axis=0).astype(np.float32)  # [3,128,256]
        WIN = WIN.reshape(3, N_STRIPS, S_ROWS, N_BLOCKS, S_COLS)
        WIN = np.ascontiguousarray(
            WIN.transpose(1, 3, 0, 2, 4).reshape(128, 3, S_ROWS, S_COLS)
        )

        in_maps.append({"xp": XP, "xp1": XP1, "win": WIN})
    return in_maps


def _gather(results):
    out = np.empty((B, C, H, W), dtype=np.float32)
    for core in range(N_CORES):
        b, half = divmod(core, 2)
        y0 = half * 128
        o = results[core]["out"].reshape(N_STRIPS, N_BLOCKS, C, S_ROWS, S_COLS)
        o = o.transpose(2, 0, 3, 1, 4).reshape(C, 128, W)
        out[b, :, y0 : y0 + 128, :] = o
    return out


def kernel(x, foa_xy, _trace=False):
    import sys

    if "/opt/trn_rl_repo" not in sys.path:
        sys.path.insert(0, "/opt/trn_rl_repo")
    from concourse.bass_utils import run_bass_kernel_spmd

    nc = _get_program()
    in_maps = _host_prep(np.asarray(x), np.asarray(foa_xy))
    res = run_bass_kernel_spmd(nc, in_maps, list(range(N_CORES)), trace=_trace)
    out = _gather(res.results)
    if _trace:
        return out, res
    return out
